# revision 45
# baseline (speedup 1.0000x reference)
"""Trainium2 Bass kernel for masked multi-head attention (b=2, n=2048, dim=1024, 16 heads).

Sharding: 8 cores = batch(2) x head-groups(4). Core c handles batch c//4 and
heads [4*(c%4), 4*(c%4)+4). Each core computes q/k/v projections for its 4
heads, device-local attention, and a partial output projection (row-parallel
to_out over its 256 inner columns). Host sums the 4 partials per batch.

v2 design notes (vs the first working version):
  - x is pre-masked and mask-sorted on the HOST and loaded ONCE ([128, DT*NL]
    bf16, NL = max(nq, nk)); queries/keys/values all read from the same SBUF
    buffer. Masked rows of x are zero, so Q-phase mask multiplies become plain
    copies, and masked-kept keys contribute exp(-30000)=0 (bias via aug row)
    or 0*v. The host-side vsall correction covers ALL masked keys' v (their
    on-device v is zero) plus the (n-nk) dropped-key count.
  - S^T layout with 65-row augmented Q/K (row 64 = mask_q / kbias) as before.
  - exp runs in [128, 3, 384] groups (3 PSUM banks per group, 2 groups in
    flight) -> 3 ACT ops per (chunk, head) instead of 5 smaller ones.
  - softmax denominator: reciprocal_approx_fast (single DVE op, ~5x faster
    than InstReciprocal, 18-bit accurate) reading po row 64 directly from
    PSUM; broadcast via rank-1 ones matmul; one DVE multiply into otp.
  - output projection + y DMA are interleaved per query chunk (emitted inside
    the next chunk's first score block) so the PE never has a serial DMA tail;
    y partials are written in bf16 (host sums in fp32).
  - engine balance: ACT = exp + K/V copies (idle during B except exp);
    DVE = Q copies + norm + yt copies; PE never idles >1us so the HAM
    clock stays at 2.4 GHz.
"""

import numpy as np

import concourse.bacc as bacc
import concourse.bass as bass
import concourse.tile as tile
from concourse import mybir
from concourse.bass_utils import run_bass_kernel_spmd

F32 = mybir.dt.float32
F32R = mybir.dt.float32r
BF16 = mybir.dt.bfloat16
EXP = mybir.ActivationFunctionType.Exp

N_CORES = 8
HEADS = 16
DH = 64
SCALE = DH ** -0.5
NEG = -30000.0


def ml_dtypes_bf16():
    import ml_dtypes
    return ml_dtypes.bfloat16


def build_nc(n=2048, d_model=1024, hl=4, nq=None, nk=None,
             interleave_c=True, fast_recip=True, eg=3, vsrow=True):
    """Build + compile the single-core Bass program (SPMD across 8 cores)."""
    dh = DH
    DT = d_model // 128      # contraction tiles for the projections
    NB = (hl * dh) // 128    # packed head blocks (2 heads each)
    HPB = 128 // dh          # heads per block = 2
    if nq is None:
        nq = n
    if nk is None:
        nk = n
    NL = max(nq, nk)         # loaded (sorted, pre-masked) x columns
    NTK = nk // 128          # kept key tiles
    qck = 384 if nq % 384 == 0 else 128
    NCHQ = nq // qck
    NTQ = nq // 128
    kck = 384 if nk % 384 == 0 else 128
    NCHK = nk // kck

    nc = bacc.Bacc("TRN2", target_bir_lowering=False, debug=False,
                   enable_asserts=False, num_devices=N_CORES)

    WROW = 3 * hl * dh
    xm_d = nc.dram_tensor("xm", [128, DT * NL], BF16, kind="ExternalInput").ap()
    wc_d = nc.dram_tensor("wcat", [128, DT * WROW + NB * d_model], BF16,
                          kind="ExternalInput").ap()
    kbh_d = nc.dram_tensor("kbiash", [1, nk], BF16, kind="ExternalInput").ap()
    mq_d = nc.dram_tensor("maskq", [1, nq], F32, kind="ExternalInput").ap()
    on_d = nc.dram_tensor("vones", [1, 64], BF16, kind="ExternalInput").ap()
    me_d = nc.dram_tensor("minv", [1, nq], BF16, kind="ExternalInput").ap()
    vs_d = nc.dram_tensor("vsall", [1, hl * (dh + 1)], BF16, kind="ExternalInput").ap()
    y_d = nc.dram_tensor("y", [nq, d_model], BF16, kind="ExternalOutput").ap()

    dmae = [nc.sync, nc.scalar]

    with tile.TileContext(nc) as tc:
        with tc.tile_pool(name="persist", bufs=1) as persist:
            ones_r = persist.tile([1, dh], BF16, tag="ones_r")
            mq_t = persist.tile([1, nq], F32, tag="mq_t")
            qa = [persist.tile([65, nq], BF16, name=f"qa{h}", tag=f"qa{h}")
                  for h in range(hl)]
            ka = [persist.tile([65, nk], BF16, name=f"ka{h}", tag=f"ka{h}")
                  for h in range(hl)]
            # vab[b]: [key-part, ktile, head-in-block, dh+1]; col dh is ones
            vab = [persist.tile([128, NTK, HPB, dh + 1], BF16,
                                name=f"vab{b}", tag=f"vab{b}")
                   for b in range(NB)]
            otp = [persist.tile([128, nq], BF16, name=f"otp{b}", tag=f"otp{b}")
                   for b in range(NB)]
            wo_all = persist.tile([128, NB, d_model], BF16, tag="wo_all")
            wo = [wo_all[:, b, :] for b in range(NB)]
            vs_t = [persist.tile([1, dh + 1], BF16, name=f"vs{h}", tag=f"vs{h}")
                    for h in range(hl)]
            me_t = persist.tile([1, nq], BF16, tag="me_t")

            # ---------------- stage A: q/k/v projections ----------------
            with tc.tile_pool(name="stA", bufs=1) as stA, \
                 tc.tile_pool(name="psA", bufs=8, space="PSUM") as psA:
                w_all = stA.tile([128, DT, 3, hl * dh], BF16, tag="w_all")
                xt_all = stA.tile([128, DT, NL], BF16, tag="xt_all")
                xt = [xt_all[:, t, :] for t in range(DT)]
                wq = [w_all[:, t, 0, :] for t in range(DT)]
                wk = [w_all[:, t, 1, :] for t in range(DT)]
                wv = [w_all[:, t, 2, :] for t in range(DT)]
                # t-interleaved input DMA on the two fast hardware DGE
                # queues (gpsimd triggers are slow ucode, ~0.64us each)
                for t in range(DT):
                    nc.scalar.dma_start(
                        out=w_all[:, t, :, :].rearrange("p a b -> p (a b)"),
                        in_=wc_d[:, t * WROW:(t + 1) * WROW])
                    nc.sync.dma_start(out=xt_all[:, t, :],
                                      in_=xm_d[:, t * NL:(t + 1) * NL])
                # w_out (needed later) + small constants
                nc.scalar.dma_start(
                    out=wo_all.rearrange("p a b -> p (a b)"),
                    in_=wc_d[:, DT * WROW:])
                nc.scalar.dma_start(out=mq_t, in_=mq_d)
                nc.scalar.dma_start(out=ones_r, in_=on_d)
                for h in range(hl):
                    nc.scalar.dma_start(out=ka[h][64:65, :], in_=kbh_d)
                nc.scalar.dma_start(out=me_t, in_=me_d)
                for h in range(hl):
                    nc.scalar.dma_start(out=vs_t[h],
                                        in_=vs_d[0:1, h * (dh + 1):(h + 1) * (dh + 1)])
                for h in range(hl):
                    nc.scalar.copy(out=qa[h][64:65, :], in_=mq_t)
                for b in range(NB):
                    nc.vector.memset(vab[b][:, :, :, dh:dh + 1], 1.0)

                # Q phase: 6 banks held, contraction t-major
                psq = {}
                for b in range(NB):
                    for j in range(NCHQ):
                        psq[b, j] = psA.tile([128, qck], F32,
                                             padded_shape=[128, 512],
                                             name=f"psq{b}_{j}", tag="psA")
                for t in range(DT):
                    for b in range(NB):
                        for j in range(NCHQ):
                            nc.tensor.matmul(
                                psq[b, j],
                                lhsT=wq[t][:, b * 128:(b + 1) * 128],
                                rhs=xt[t][:, j * qck:(j + 1) * qck],
                                start=(t == 0), stop=(t == DT - 1))
                for b in range(NB):
                    for j in range(NCHQ):
                        cs = slice(j * qck, (j + 1) * qck)
                        for l in range(HPB):
                            h = b * HPB + l
                            nc.scalar.copy(
                                out=qa[h][0:dh, cs],
                                in_=psq[b, j][l * dh:(l + 1) * dh, :])
                # K phase
                psk = {}
                for b in range(NB):
                    for j in range(NCHK):
                        psk[b, j] = psA.tile([128, kck], F32,
                                             padded_shape=[128, 512],
                                             name=f"psk{b}_{j}", tag="psA")
                for t in range(DT):
                    for b in range(NB):
                        for j in range(NCHK):
                            nc.tensor.matmul(
                                psk[b, j],
                                lhsT=wk[t][:, b * 128:(b + 1) * 128],
                                rhs=xt[t][:, j * kck:(j + 1) * kck],
                                start=(t == 0), stop=(t == DT - 1))
                for b in range(NB):
                    for j in range(NCHK):
                        cs = slice(j * kck, (j + 1) * kck)
                        for l in range(HPB):
                            h = b * HPB + l
                            nc.scalar.copy(out=ka[h][0:dh, cs],
                                           in_=psk[b, j][l * dh:(l + 1) * dh, :])
                # V phase: rings of <=8 one-bank tiles
                for half in range((NTK + 7) // 8):
                    cnt = min(8, NTK - half * 8)
                    psv = {}
                    for i in range(cnt):
                        psv[i] = psA.tile([128, hl * dh], F32,
                                          padded_shape=[128, 512],
                                          name=f"psv{i}", tag="psA")
                    for d in range(DT):
                        for i in range(cnt):
                            t = half * 8 + i
                            nc.tensor.matmul(
                                psv[i],
                                lhsT=xt[d][:, t * 128:(t + 1) * 128],
                                rhs=wv[d],
                                start=(d == 0), stop=(d == DT - 1))
                    for i in range(cnt):
                        t = half * 8 + i
                        for b in range(NB):
                            eng = nc.scalar if b == 0 else nc.vector
                            (eng.copy if b == 0 else eng.tensor_copy)(
                                out=vab[b][:, t, :, 0:dh],
                                in_=psv[i][:, b * 128:(b + 1) * 128]
                                .rearrange("p (l d) -> p l d", l=HPB))
                if vsrow:
                    # the last key slot (nk-1) is always a masked key (its
                    # premasked k/v are zero, and exp of its logits is exactly
                    # 1 for masked queries / 0 for unmasked) -> writing the
                    # host-side v correction row here replaces a per-chunk
                    # K=1 correction matmul in the attention loop
                    for h in range(hl):
                        b, l = divmod(h, HPB)
                        nc.sync.dma_start(
                            out=vab[b][127:128, NTK - 1, l, :],
                            in_=vs_d[0:1, h * (dh + 1):(h + 1) * (dh + 1)])

            # ---------------- stage B: attention + interleaved stage C ----
            NG = (NTK + eg - 1) // eg    # exp groups of up to eg key tiles
            with tc.tile_pool(name="eb", bufs=6) as eb, \
                 tc.tile_pool(name="nrm", bufs=3) as nrm, \
                 tc.tile_pool(name="ytp", bufs=3) as ytp, \
                 tc.tile_pool(name="pss", bufs=2, space="PSUM") as pss, \
                 tc.tile_pool(name="pso", bufs=2, space="PSUM") as pso:

                def emit_out_chunk(j):
                    """Output projection + y DMA for query chunk j."""
                    t0, t1 = j * qck // 128, (j + 1) * qck // 128
                    for t in range(t0, t1):
                        yc = pss.tile([128, 2, 512], F32, name=f"yc{t}",
                                      tag="pss")
                        for half in range(2):
                            for b in range(NB):
                                nc.tensor.matmul(
                                    yc[:, half, :],
                                    lhsT=otp[b][:, t * 128:(t + 1) * 128],
                                    rhs=wo[b][:, half * 512:(half + 1) * 512],
                                    start=(b == 0), stop=(b == NB - 1))
                        yt = ytp.tile([128, 2, 512], BF16, name="yt", tag="yt")
                        nc.vector.tensor_copy(out=yt, in_=yc)
                        dmae[t % 2].dma_start(
                            out=y_d[t * 128:(t + 1) * 128, :],
                            in_=yt.rearrange("p a b -> p (a b)"))

                def emit_pv(prev):
                    """Deferred PV + denominator chain for the previous
                    iteration: its exps are long done, so these matmuls slot
                    between the current iteration's score groups and keep the
                    PE dense (HAM stays at full clock)."""
                    b, l, cs, ets = prev
                    po = pso.tile([65, qck], F32,
                                  padded_shape=[65, 512], name="po",
                                  tag="pso")
                    for t in range(NTK):
                        nc.tensor.matmul(
                            po,
                            lhsT=vab[b][:, t, l, :],
                            rhs=ets[t // eg][:, t % eg, :],
                            start=(t == 0),
                            stop=(t == NTK - 1 and vsrow))
                    if not vsrow:
                        nc.tensor.matmul(
                            po, lhsT=vs_t[b * HPB + l], rhs=me_t[0:1, cs],
                            start=False, stop=True)
                    # denominator row -> SBUF, fast reciprocal (SBUF->SBUF;
                    # its bitwise seed breaks on PSUM reads), bf16 cast so
                    # the broadcast matmul is single-pass
                    drs = nrm.tile([1, qck], F32, name="drs", tag="drs")
                    nc.vector.tensor_copy(out=drs, in_=po[64:65, :])
                    rsb = nrm.tile([1, qck], F32, name="rsb", tag="rsb")
                    if fast_recip:
                        nc.vector.reciprocal_approx_fast(out=rsb, in_=drs)
                    else:
                        nc.vector.reciprocal(out=rsb, in_=drs)
                    rsc = nrm.tile([1, qck], BF16, name="rsc", tag="rsc")
                    nc.vector.tensor_copy(out=rsc, in_=rsb)
                    return (b, l, cs, po, rsc)

                def emit_norm_tail(pend):
                    """Broadcast 1/den (bf16, single PE pass) and scale po
                    rows 0..63 into otp; two iterations behind the scores."""
                    b, l, cs, po, rsc = pend
                    bb = pso.tile([dh, qck], F32,
                                  padded_shape=[dh, 512], name="bb",
                                  tag="pso")
                    nc.tensor.matmul(bb, lhsT=ones_r, rhs=rsc,
                                     start=True, stop=True)
                    bbs = nrm.tile([dh, qck], F32, name="bbs", tag="bbs")
                    nc.vector.tensor_copy(out=bbs, in_=bb)
                    nc.vector.tensor_mul(
                        out=otp[b][l * dh:(l + 1) * dh, cs],
                        in0=po[0:dh, :], in1=bbs)

                prev = None     # iteration awaiting PV
                pend = None     # iteration awaiting normalize tail
                for j in range(NCHQ):
                    cs = slice(j * qck, (j + 1) * qck)
                    for h in range(hl):
                        b, l = divmod(h, HPB)
                        ets = []
                        for g in range(NG):
                            w = min(eg, NTK - eg * g)
                            ps = pss.tile([128, w, qck], F32,
                                          padded_shape=[128, w, 512],
                                          name=f"ps{g}", tag="pss")
                            for u in range(w):
                                t = eg * g + u
                                nc.tensor.matmul(
                                    ps[:, u, :],
                                    lhsT=ka[h][:, t * 128:(t + 1) * 128],
                                    rhs=qa[h][:, cs],
                                    start=True, stop=True)
                            et = eb.tile([128, w, qck], BF16, name="et",
                                         tag="et")
                            nc.scalar.activation(out=et, in_=ps, func=EXP)
                            ets.append(et)
                            # previous iterations' normalize tail + PV ride
                            # between score groups so the PE never waits on
                            # this exp (norm first: frees the po ring slot)
                            if g == 1:
                                if pend is not None:
                                    emit_norm_tail(pend)
                                    pend = None
                                if prev is not None:
                                    pend = emit_pv(prev)
                                    prev = None
                        if interleave_c and h == 1 and j > 0:
                            emit_out_chunk(j - 1)
                        prev = (b, l, cs, ets)
                if pend is not None:
                    emit_norm_tail(pend)
                emit_norm_tail(emit_pv(prev))
                if interleave_c:
                    emit_out_chunk(NCHQ - 1)
                else:
                    for j in range(NCHQ):
                        emit_out_chunk(j)

    nc.compile()
    return nc


_NC_CACHE = {}
BUILD_OPTS = {}


def _get_nc(n=2048, d_model=1024, hl=4, nq=None, nk=None, vsrow=True):
    opts = dict(BUILD_OPTS)
    opts.setdefault("vsrow", vsrow)
    key = (n, d_model, hl, nq, nk, tuple(sorted(opts.items())))
    if key not in _NC_CACHE:
        _NC_CACHE[key] = build_nc(n, d_model, hl, nq=nq, nk=nk, **opts)
    return _NC_CACHE[key]


def _pick_nq(mask, n):
    """Kept-query count: smallest multiple of 384 covering max(m1)+1."""
    m1max = int(np.asarray(mask).astype(bool).sum(axis=1).max())
    nq = ((m1max + 1 + 383) // 384) * 384
    return min(nq, n)


def _pick_nk(mask, n):
    """Kept-key count: smallest multiple of 128 covering max(m1)."""
    m1max = int(np.asarray(mask).astype(bool).sum(axis=1).max())
    nk = ((m1max + 127) // 128) * 128
    return min(nk, n)


def make_in_maps(x, mask, w_qkv, w_out, nq=None, nk=None):
    """Host-side sharding: per-core input dict."""
    x = np.asarray(x, dtype=np.float32)
    mask = np.asarray(mask)
    w_qkv = np.asarray(w_qkv, dtype=np.float32)
    w_out = np.asarray(w_out, dtype=np.float32)
    b, n, dim = x.shape
    inner = HEADS * DH
    hl = HEADS // 4                      # 4 heads per core
    hw = hl * DH                         # 256 inner cols per core
    import ml_dtypes
    bf16 = ml_dtypes.bfloat16
    maskf = mask.astype(np.float32)
    ones64 = np.ones((1, 64), ml_dtypes_bf16())
    in_maps = []
    DT = dim // 128
    hw3 = 3 * hw
    if nq is None:
        nq = _pick_nq(mask, n)
    if nk is None:
        nk = _pick_nk(mask, n)
    NL = max(nq, nk)
    orders = [np.argsort(-maskf[bc], kind="stable") for bc in range(b)]
    for c in range(N_CORES):
        bc, hg = divmod(c, 4)
        rs = slice(hg * hw, (hg + 1) * hw)
        wq = w_qkv[0 * inner:1 * inner, :][rs, :]
        wk = w_qkv[1 * inner:2 * inner, :][rs, :]
        wv = w_qkv[2 * inner:3 * inner, :][rs, :]
        ms = maskf[bc][orders[bc]]
        # pre-masked, mask-sorted x: kept rows first, masked rows zeroed
        xms = x[bc][orders[bc][:NL], :] * ms[:NL, None]
        xm = (xms.T.reshape(DT, 128, NL).transpose(1, 0, 2)
              .reshape(128, DT * NL).astype(bf16))
        mqs = ms[:nq]
        # v correction for masked queries: sum of v over ALL masked keys
        # (their on-device v is zero), plus the dropped-key count (n - nk)
        xmasked = x[bc][maskf[bc] == 0, :].sum(axis=0)    # [dim]
        vsall = np.zeros((1, hl * (DH + 1)), np.float32)
        for lh in range(hl):
            wv_h = wv[lh * DH:(lh + 1) * DH, :]           # [64, dim]
            vsall[0, lh * (DH + 1):lh * (DH + 1) + DH] = xmasked @ wv_h.T
            vsall[0, lh * (DH + 1) + DH] = n - nk
        wstk = np.stack([(wq.T * np.float32(SCALE)).reshape(DT, 128, hw),
                         wk.T.reshape(DT, 128, hw),
                         wv.T.reshape(DT, 128, hw)], axis=2)  # [DT,128,3,hw]
        wflat = wstk.transpose(1, 0, 2, 3).reshape(128, DT * hw3)
        NB = hw // 128
        wop = (w_out[:, rs].T.reshape(NB, 128, dim).transpose(1, 0, 2)
               .reshape(128, NB * dim))
        wcat = np.concatenate([wflat, wop], axis=1).astype(bf16)
        in_maps.append({
            "xm": np.ascontiguousarray(xm),
            "wcat": np.ascontiguousarray(wcat),
            "kbiash": ((ms[:nk] - 1.0) * np.float32(-NEG)
                       ).reshape(1, nk).astype(bf16),
            "maskq": mqs.reshape(1, nq).astype(np.float32),
            "minv": (1.0 - mqs).reshape(1, nq).astype(bf16),
            "vsall": vsall.astype(bf16),
            "vones": ones64,
        })
    return in_maps


def gather(results, mask, b=2, n=2048, dim=1024, nq=None):
    """Sum the 4 head-group partials per batch and undo the query sort.

    All fully-masked queries share one output row (uniform attention over all
    keys), so positions beyond the kept set copy the first masked kept row."""
    maskf = np.asarray(mask).astype(np.float32)
    if nq is None:
        nq = _pick_nq(mask, n)
    y = np.zeros((b, n, dim), dtype=np.float32)
    for bc in range(b):
        yk = np.zeros((nq, dim), dtype=np.float32)
        for c in range(N_CORES):
            if c // 4 == bc:
                yk += results[c]["y"].astype(np.float32)
        order = np.argsort(-maskf[bc], kind="stable")
        m1 = int(maskf[bc].sum())
        y[bc][order[:nq]] = yk
        if nq < n:
            y[bc][order[nq:]] = yk[m1]
    return y


def run(x, mask, w_qkv, w_out, trace=False, trace_cores=None):
    b, n, dim = np.asarray(x).shape
    nq = _pick_nq(mask, n)
    nk = _pick_nk(mask, n)
    m1max = int(np.asarray(mask).astype(bool).sum(axis=1).max())
    nc = _get_nc(n=n, d_model=dim, hl=HEADS // 4, nq=nq, nk=nk,
                 vsrow=(nk > m1max))
    in_maps = make_in_maps(x, mask, w_qkv, w_out, nq=nq, nk=nk)
    res = run_bass_kernel_spmd(nc, in_maps, core_ids=list(range(N_CORES)),
                               trace=trace, trace_cores=trace_cores)
    return gather(res.results, mask, b=b, n=n, dim=dim, nq=nq), res


def kernel(x, mask, w_qkv, w_out):
    y, _ = run(x, mask, w_qkv, w_out)
    return y
